# revision 14
# baseline (speedup 1.0000x reference)
"""Trainium2 Bass kernel for a CGNS block (GNN message passing).

Math: the reference builds A = a a^T + I (rank-1 + identity), L = D^-1/2 A D^-1/2,
then out = relu(BN(conv1x1(cat[x@A, (L@x^T)^T]))).  Exploiting the rank-1
structure, with a = relu(tanh(w)), S = sum(a), d_n = 1/sqrt(a_n*S + 1),
u = d*a, s0 = x@a, s1 = x@u, the whole block collapses to

  y[:, n] = W1~ x[:, n] + d2[n] * (W2~ x[:, n]) + a[n] v1 + u[n] v2 + b~
  out     = relu(y)

where W~ are the BN-folded conv weights, v1 = W1~ s0, v2 = W2~ s1.  No [N,N]
matrix is ever materialized.

Sharding: 8 cores; core i handles batch b = i//2, half h = i%2 of the N=4096
node dim (2048 columns each).  Each core reads the full x[b] once in
transposed layout (for the s0/s1 reduction, which needs all of N) and its own
half in natural layout (for the main matmuls).  n-chunks are rolled per-core
so that chunks 0..15 are always the core's own half -> identical SPMD program.

v4 structure (all matmul-path data bf16; tolerance is 2e-2 so bf16's ~0.4%
noise is fine and PE runs 4x faster than fp32's 4-cycles/row):
 - q = W2~x accumulates into its OWN PSUM tiles whose groups close right
   after mm_q, so the qd2 = q*d2 evacuation overlaps all later matmul work
   (the framework orders PSUM reads after the owning group's stop).
 - y1 accumulates as mm_y (x+ones stationary vs W1~T+[b~] moving; gated
   only by early DMAs) plus one K=2 rank-2 accumulation per chunk
   (a/u rows x v1/v2 rows, all on partitions 0/1 - no partition-64 row
   DMAs and no wAB v-row DMA roundtrip, which stalled the PE before).
 - a/u rows come from two PE transposes of the column-layout values, not a
   second scalar-engine chain (no extra ACT_TABLE_LOADs).
 - front-end: sqrt(t) on Scalar runs in parallel with 1/t on Vector, then
   u = a/sqrt(t) is a single divide.
 - epilogue: yo = relu(y1(PSUM) + qd2(SBUF)) - one PSUM operand per TT,
   relus on Scalar, out-DMA enqueues alternate the SP/Activation queues.
   GpSimd (slow ucode engine) only enqueues DMAs and builds constants.
"""

import numpy as np

import concourse.bacc as bacc
import concourse.bass as bass
import concourse.tile as tile
from concourse import masks, mybir

FP = mybir.dt.float32
BF = mybir.dt.bfloat16
B, C, N = 4, 64, 4096
NH = N // 2          # columns per core
JH = NH // 128       # 16 chunks per core half
JF = N // 128        # 32 chunks full N
BN_EPS = 1e-5


def build_nc():
    # Bacc (not raw Bass): its compile() pipeline legalizes TRN2's
    # one-wait-per-instruction constraint (move_matmul_waits_to_ldweights,
    # generate_event_semaphores) which Tile-emitted multi-waits require.
    nc = bacc.Bacc()
    AF = mybir.ActivationFunctionType
    OP = mybir.AluOpType

    # DRAM I/O (per-core shards supplied via in_maps)
    xt = nc.dram_tensor("xt", [128, JF, C], BF, kind="ExternalInput")
    xh = nc.dram_tensor("xh", [C, NH], BF, kind="ExternalInput")
    wcol = nc.dram_tensor("wcol", [128, 32], FP, kind="ExternalInput")
    wv = nc.dram_tensor("wv", [C, 2 * C], BF, kind="ExternalInput")
    brow1 = nc.dram_tensor("brow1", [1, 2 * C], BF, kind="ExternalInput")
    out = nc.dram_tensor("out", [128, JH, C], BF, kind="ExternalOutput")

    with tile.TileContext(nc) as tc:
        with (
            tc.tile_pool(name="sb", bufs=1) as sb,
            tc.tile_pool(name="ps", bufs=1, space="PSUM") as ps,
        ):
            # SBUF tiles
            xt_sb = sb.tile([128, JF, C], BF, name="xt_sb")
            xa = sb.tile([65, NH], BF, name="xa")        # x half + ones row
            wcol_sb = sb.tile([128, 32], FP, name="wcol_sb")
            wAB = sb.tile([65, 2 * C], BF, name="wAB")   # [W1~T|W2~T] + [b~|0]
            ones = sb.tile([128, 128], FP, name="ones")
            ident = sb.tile([128, 128], BF, name="ident")
            ones16 = sb.tile([JH, 128], BF, name="ones16")
            rowsb = sb.tile([JH, 2 * 128], BF, name="rowsb")  # a/u rows
            tcol = sb.tile([128, 32], FP, name="tcol")
            acol = sb.tile([128, 32], FP, name="acol")
            ttile = sb.tile([128, 32], FP, name="ttile")
            stcol = sb.tile([128, 32], FP, name="stcol")
            d2col = sb.tile([128, 32], FP, name="d2col")
            apart = sb.tile([128, 1], FP, name="apart")
            sS = sb.tile([128, 1], FP, name="sS")
            au = sb.tile([128, 2 * 32], BF, name="au")   # a/u interleaved
            s01a = sb.tile([C, 2], BF, name="s01a")      # [s0 | 0]
            s01b = sb.tile([C, 2], BF, name="s01b")      # [0 | s1]
            aurow2 = sb.tile([2, NH], BF, name="aurow2")  # a row / u row
            vvt = sb.tile([2, C], BF, name="vvt")        # v1 / v2 rows
            qd2 = sb.tile([128, JH * C], BF, name="qd2")
            yo = sb.tile([128, JH * C], BF, name="yo")

            # PSUM tiles (each padded to a bank; 8 total = 8 banks).  q gets
            # its own tiles so its accumulation groups close right after
            # mm_q and the qd2 reads can overlap later matmul work.
            p_sm = ps.tile([128, 1], FP, name="p_sm")
            p_s = ps.tile([C, 2], FP, name="p_s")
            p_v = ps.tile([2, C], FP, name="p_v")
            p_t = ps.tile([JH, 2 * 128], BF, name="p_t")
            p_y = [ps.tile([128, 512], FP, name=f"p_y_{t}") for t in range(2)]
            p_q = [ps.tile([128, 512], FP, name=f"p_q_{t}") for t in range(2)]

            # ---- DMAs in, by criticality per queue.  SP: wcol (gates the
            # scalar chain) then xh/wv/brow1 (gates of mm_q/mm_y).  The xt
            # halves (needed later, by s0/s1) ride the Activation and SWDGE
            # queues.
            nc.sync.dma_start(wcol_sb[:], wcol[:])
            nc.scalar.dma_start(xt_sb[:, 0:8, :], xt[:, 0:8, :])
            nc.gpsimd.dma_start(xt_sb[:, 16:24, :], xt[:, 16:24, :])
            nc.sync.dma_start(xa[0:C, 0:1024], xh[:, 0:1024])
            nc.scalar.dma_start(xt_sb[:, 8:16, :], xt[:, 8:16, :])
            nc.gpsimd.dma_start(xt_sb[:, 24:32, :], xt[:, 24:32, :])
            nc.sync.dma_start(xa[0:C, 1024:2048], xh[:, 1024:2048])
            nc.sync.dma_start(wAB[0:C, :], wv[:])
            nc.sync.dma_start(wAB[64:65, :], brow1[:])

            # constants: fp32 ones (S broadcast), bf16 identity (PE
            # transpose), bf16 ones row -> xa row 64 via DMA (engine writes
            # at partition 64+ hang HW; DMA has no partition restrictions).
            nc.gpsimd.memset(ones16[:], 1.0)
            nc.gpsimd.dma_start(xa[64:65, :], ones16[:])
            masks.make_identity(nc, ident[:])
            nc.vector.memset(ones[:], 1.0)
            nc.vector.memset(s01a[:], 0.0)
            nc.vector.memset(s01b[:], 0.0)

            # ---- scalar/vector front-end (column layout, fp32):
            # a = relu(tanh(w)) with the partial row-sum fused via accum_out;
            # S broadcast to all partitions via ones-matmul; t = a*S + 1;
            # then sqrt(t) on Scalar in parallel with 1/t on Vector, and
            # u = a/sqrt(t) via one divide straight into the bf16 a/u tile.
            nc.scalar.activation(tcol[:], wcol_sb[:], AF.Tanh)
            nc.scalar.activation(acol[:], tcol[:], AF.Relu, accum_out=apart[:])
            nc.tensor.matmul(p_sm[:], ones[:], apart[:], start=True, stop=True)
            au_v = au[:].rearrange("p (k t) -> p k t", t=2)
            nc.vector.tensor_copy(au_v[:, :, 0], acol[:])
            nc.vector.tensor_copy(sS[:], p_sm[:])
            nc.vector.tensor_scalar(
                ttile[:], acol[:], sS[:], 1.0, op0=OP.mult, op1=OP.add
            )
            nc.scalar.sqrt(stcol[:], ttile[:])
            nc.vector.reciprocal(d2col[:], ttile[:])
            # d = sqrt(t)/t = t^-1/2 (no DVE divide; sqrt/recip ran parallel)
            nc.vector.tensor_mul(stcol[:], stcol[:], d2col[:])
            nc.vector.tensor_mul(au_v[:, :, 1], acol[:], stcol[:])

            # a/u row layout via PE transpose of the own-half columns
            # (chunks 0..15 are the core's own half by construction).
            nc.tensor.transpose(p_t[:, 0:128], au_v[:, 0:JH, 0], ident[:])

            # ---- mm_q: q = W2~ x per chunk, own PSUM tiles, closed early.
            for j in range(JH):
                t, jj = divmod(j, 8)
                nc.tensor.matmul(
                    p_q[t][:, C * jj : C * (jj + 1)],
                    xa[0:C, 128 * j : 128 * (j + 1)],
                    wAB[0:C, C : 2 * C],
                    start=(jj == 0), stop=(jj == 7),
                    skip_group_check=True,
                )

            # mm_y: y1 = W1~ x + b~ (ones row x [b~|0] row).  One start=True
            # per PSUM bank: start marks the whole 2KB zero-region
            # pending-zero, so a second start on the same bank would turn
            # later accumulations into overwrites.
            def mm_y(j):
                t, jj = divmod(j, 8)
                nc.tensor.matmul(
                    p_y[t][:, C * jj : C * (jj + 1)],
                    xa[:, 128 * j : 128 * (j + 1)],
                    wAB[:, 0:C],
                    start=(jj == 0), stop=False,
                    skip_group_check=True,
                )

            for j in range(8):
                mm_y(j)

            nc.tensor.transpose(p_t[:, 128:256], au_v[:, 0:JH, 1], ident[:])
            nc.vector.tensor_copy(rowsb[:], p_t[:])
            # flatten the [16,128] row-staging into true [1, 2048] rows
            # (partition-crossing, so DMA) - off the PE critical path.
            nc.gpsimd.dma_start(aurow2[0:1, :], rowsb[:, 0:128])
            nc.gpsimd.dma_start(aurow2[1:2, :], rowsb[:, 128:256])

            # ---- s0/s1 reduction over full N (PE, accumulate in PSUM) ----
            for j in range(JF):
                nc.tensor.matmul(
                    p_s[:],
                    xt_sb[:, j, :],
                    au[:, 2 * j : 2 * j + 2],
                    start=(j == 0),
                    stop=(j == JF - 1),
                )
            nc.vector.tensor_copy(s01a[:, 0:1], p_s[:, 0:1])
            nc.vector.tensor_copy(s01b[:, 1:2], p_s[:, 1:2])

            for j in range(8, JH):
                mm_y(j)

            # v1/v2 stacked on partitions 0/1 via zero-padded stationaries:
            # [s0|0]^T W1~T -> [v1;0], [0|s1]^T W2~T -> [0;v2], accumulated.
            nc.tensor.matmul(
                p_v[:], s01a[:], wAB[0:C, 0:C], start=True, stop=False,
                skip_group_check=True,
            )
            nc.tensor.matmul(
                p_v[:], s01b[:], wAB[0:C, C : 2 * C], start=False, stop=True,
                skip_group_check=True,
            )
            nc.vector.tensor_copy(vvt[:], p_v[:])

            # ---- rank-2 term: one K=2 accumulation per chunk into y1:
            # a[128j+m]*v1[c] + u[128j+m]*v2[c].
            for j in range(JH):
                t, jj = divmod(j, 8)
                nc.tensor.matmul(
                    p_y[t][:, C * jj : C * (jj + 1)],
                    aurow2[:, 128 * j : 128 * (j + 1)],
                    vvt[:],
                    start=False, stop=(jj == 7), skip_group_check=True,
                )

            # ---- epilogue: yo = relu(q*d2 + y1).  qd2 per q-tile runs as
            # soon as mm_q closes (early, overlapping the other matmuls);
            # the adds pair y1 (PSUM - one PSUM operand per TT) with qd2
            # (SBUF).  Relus on Scalar; out-DMA enqueues alternate SP /
            # Activation so they don't serialize at the tail.
            for t in range(2):
                ts_ = slice(512 * t, 512 * (t + 1))
                nc.vector.tensor_tensor(
                    qd2[:, ts_].rearrange("p (j c) -> p j c", c=C),
                    p_q[t][:].rearrange("p (j c) -> p j c", c=C),
                    d2col[:, 8 * t : 8 * (t + 1), None].broadcast_to((128, 8, C)),
                    op=OP.mult,
                )
            for g in range(4):
                gs = slice(256 * g, 256 * (g + 1))
                nc.vector.tensor_tensor(
                    yo[:, gs],
                    p_y[g // 2][:, 256 * (g % 2) : 256 * (g % 2 + 1)],
                    qd2[:, gs],
                    op=OP.add,
                )
                nc.scalar.activation(yo[:, gs], yo[:, gs], AF.Relu)
                eng = nc.sync if g % 2 == 0 else nc.scalar
                eng.dma_start(
                    out[:, 4 * g : 4 * (g + 1), :],
                    yo[:, gs].rearrange("p (j c) -> p j c", c=C),
                )
    nc.compile()
    return nc


def make_in_maps(x, w, conv_w, conv_b, bn_gamma, bn_beta, bn_mean, bn_var):
    import ml_dtypes

    bf16 = ml_dtypes.bfloat16
    x = np.asarray(x, np.float32)
    w = np.asarray(w, np.float32)
    conv_w = np.asarray(conv_w, np.float32)
    conv_b = np.asarray(conv_b, np.float32)
    bn_gamma = np.asarray(bn_gamma, np.float32)
    bn_beta = np.asarray(bn_beta, np.float32)
    bn_mean = np.asarray(bn_mean, np.float32)
    bn_var = np.asarray(bn_var, np.float32)

    scale = bn_gamma / np.sqrt(bn_var + BN_EPS)
    wmat = conv_w * scale[:, None]                       # [64, 128] BN-folded
    w1t = np.ascontiguousarray(wmat[:, :C].T)            # [c, o]
    w2t = np.ascontiguousarray(wmat[:, C:].T)
    wv = np.ascontiguousarray(
        np.concatenate([w1t, w2t], axis=1).astype(bf16)
    )
    brow1 = np.zeros((1, 2 * C), np.float32)
    brow1[0, :C] = conv_b * scale + bn_beta - bn_mean * scale
    brow1 = brow1.astype(bf16)

    in_maps = []
    for i in range(8):
        b, h = divmod(i, 2)
        xb = x[b, :, :, 0]                               # [64, 4096]
        order = np.roll(np.arange(JF), -JH * h)          # own half first
        xt_jpc = np.ascontiguousarray(xb.T).reshape(JF, 128, C)
        xt_pjc = np.ascontiguousarray(
            xt_jpc[order].transpose(1, 0, 2).astype(bf16)
        )
        xhb = np.ascontiguousarray(xb[:, NH * h : NH * (h + 1)].astype(bf16))
        wcol = np.ascontiguousarray(w[b].reshape(JF, 128).T[:, order])
        in_maps.append(
            {
                "xt": xt_pjc,
                "xh": xhb,
                "wcol": wcol,
                "wv": wv,
                "brow1": brow1,
            }
        )
    return in_maps


def assemble_out(results):
    out = np.empty((B, C, N), np.float32)
    for i in range(8):
        b, h = divmod(i, 2)
        blk = np.asarray(results[i]["out"]).astype(np.float32)  # [128, 16, 64]
        y_half = blk.transpose(1, 0, 2).reshape(NH, C)   # row = 128*j + p
        out[b, :, NH * h : NH * (h + 1)] = y_half.T
    return out[..., None]


_NC = None


def kernel(**inputs):
    global _NC
    from concourse.bass_utils import run_bass_kernel_spmd

    if _NC is None:
        _NC = build_nc()
    in_maps = make_in_maps(**inputs)
    res = run_bass_kernel_spmd(_NC, in_maps, list(range(8)))
    return assemble_out(res.results)
